# revision 1
# baseline (speedup 1.0000x reference)
"""Expert-parallel SwiGLU MLP (MoE experts) for 8 Trainium2 NeuronCores.

Problem: routed_in_egD [E*G, D] fp32, w1/w3 [E, D, F], w2 [E, F, D], E=8,
G=2048, D=2048, F=5632.  reference:
    x_egD = routed.reshape(E, G, D)
    mid   = silu(x @ w1) * (x @ w3)          # [E, G, F]
    out   = (mid @ w2).reshape(E*G, D)

Sharding: expert-parallel — core e gets expert e's x slice + weights; no
collectives.  Each core runs three 2048x2048x5632-class GEMMs (~142 GFLOP).

Per-core kernel (all matmuls fp32r unless noted):
  phase 0: PE-transpose x [G,D] -> xT [D,G] resident in SBUF (fp32r).
  phase 1: for each f-chunk (128 rows of F): gateT/upT = w1/w3-chunk.T @ x
           accumulated over D in PSUM; SwiGLU on ACT+DVE; midT [F,G] spilled
           to DRAM as bf16.
  phase 2: out[g,d] = sum_f midT[f,g]*w2[f,d]: mid chunks stationary (bf16),
           w2 streamed (DMA-cast fp32->bf16), PSUM accumulation over F.
           Output written in natural [G, D] layout.
"""

import numpy as np

import concourse.mybir as mybir
import concourse.tile as tile
from concourse import bacc
from concourse.bass_utils import run_bass_kernel_spmd
from concourse.masks import make_identity

E, G, D, F = 8, 2048, 2048, 5632
P = 128
DO = D // P      # 16 d-chunks
FC = F // P      # 44 f-chunks
GO = G // P      # 16 g-chunks

F32 = mybir.dt.float32
F32R = mybir.dt.float32r
BF16 = mybir.dt.bfloat16


def build_nc():
    nc = bacc.Bacc("TRN2", target_bir_lowering=False)
    x = nc.dram_tensor("x", [G, D], F32, kind="ExternalInput").ap()
    w1 = nc.dram_tensor("w1", [D, F], F32, kind="ExternalInput").ap()
    w2 = nc.dram_tensor("w2", [F, D], F32, kind="ExternalInput").ap()
    w3 = nc.dram_tensor("w3", [D, F], F32, kind="ExternalInput").ap()
    out = nc.dram_tensor("out", [G, D], F32, kind="ExternalOutput").ap()

    w1r = w1.rearrange("(do p) f -> p do f", p=P)
    w3r = w3.rearrange("(do p) f -> p do f", p=P)
    w2r = w2.rearrange("(fo p) d -> p fo d", p=P)

    with tile.TileContext(nc) as tc:
        with tc.tile_pool(name="dram", bufs=1, space="DRAM") as dram:
            mid = dram.tile([F, G], BF16)
            mid_r = mid.rearrange("(fo p) g -> p fo g", p=P)

            with tc.tile_pool(name="xtp", bufs=1) as xtp:
                xT = xtp.tile([P, DO, G], F32R)

                # ---- phase 0: x [G, D] -> xT [d_in, d_out, g] (fp32r)
                with (
                    tc.tile_pool(name="p0", bufs=2) as p0,
                    tc.tile_pool(name="idp", bufs=1) as idp,
                    tc.tile_pool(name="p0ps", bufs=4, space="PSUM") as p0ps,
                ):
                    ident = idp.tile([P, P], F32)
                    make_identity(nc, ident)
                    for go in range(GO):
                        xs = p0.tile([P, D], F32, tag="xs")
                        nc.sync.dma_start(xs, x[go * P : (go + 1) * P, :])
                        for d4 in range(DO // 4):
                            tp = p0ps.tile([P, 4, P], F32, tag="tp")
                            for j in range(4):
                                do = d4 * 4 + j
                                nc.tensor.transpose(
                                    tp[:, j], xs[:, do * P : (do + 1) * P], ident
                                )
                            nc.vector.tensor_copy(
                                xT[:, d4 * 4 : (d4 + 1) * 4, go * P : (go + 1) * P],
                                tp,
                            )

                # ---- phase 1: midT[f, g] = silu(w1.T x) * (w3.T x), spill bf16
                with (
                    tc.tile_pool(name="wp", bufs=2) as wp,
                    tc.tile_pool(name="sp", bufs=2) as sp,
                    tc.tile_pool(name="mp", bufs=3) as mp,
                    tc.tile_pool(name="ps1", bufs=2, space="PSUM") as ps1,
                ):
                    for fc in range(FC):
                        w1t = wp.tile([P, DO, P], F32R, tag="w1")
                        nc.gpsimd.dma_start(w1t, w1r[:, :, fc * P : (fc + 1) * P])
                        w3t = wp.tile([P, DO, P], F32R, tag="w3")
                        nc.gpsimd.dma_start(w3t, w3r[:, :, fc * P : (fc + 1) * P])
                        for gh in range(2):
                            pg = ps1.tile([P, 2, 512], F32, tag="pg")
                            pu = ps1.tile([P, 2, 512], F32, tag="pu")
                            for d in range(DO):
                                st, sp_ = (d == 0), (d == DO - 1)
                                for j in range(2):
                                    gsl = slice((gh * 2 + j) * 512, (gh * 2 + j + 1) * 512)
                                    nc.tensor.matmul(
                                        pg[:, j], w1t[:, d], xT[:, d, gsl],
                                        start=st, stop=sp_,
                                    )
                                    nc.tensor.matmul(
                                        pu[:, j], w3t[:, d], xT[:, d, gsl],
                                        start=st, stop=sp_,
                                    )
                            tmp = sp.tile([P, 2, 512], F32, tag="tmp")
                            nc.scalar.activation(
                                tmp, pg, mybir.ActivationFunctionType.Silu
                            )
                            mo = mp.tile([P, 2, 512], BF16, tag="mo")
                            nc.vector.tensor_mul(mo, tmp, pu)
                            nc.sync.dma_start(
                                mid[fc * P : (fc + 1) * P, gh * 1024 : (gh + 1) * 1024],
                                mo,
                            )

            # ---- phase 2: out[g, d] = midT.T @ w2 (bf16 x bf16, fp32 psum)
            with (
                tc.tile_pool(name="w2p", bufs=2) as w2p,
                tc.tile_pool(name="mqp", bufs=2) as mqp,
                tc.tile_pool(name="op", bufs=3) as op,
                tc.tile_pool(name="ps2", bufs=4, space="PSUM") as ps2,
            ):
                for dq in range(4):
                    w2q = w2p.tile([P, FC, 512], BF16, tag="w2q")
                    nc.gpsimd.dma_start(w2q, w2r[:, :, dq * 512 : (dq + 1) * 512])
                    for gp in range(8):
                        mq = mqp.tile([P, FC, 256], BF16, tag="mq")
                        nc.sync.dma_start(mq, mid_r[:, :, gp * 256 : (gp + 1) * 256])
                        po0 = ps2.tile([P, 512], F32, tag="po0")
                        po1 = ps2.tile([P, 512], F32, tag="po1")
                        for fo in range(FC):
                            st, sp_ = (fo == 0), (fo == FC - 1)
                            nc.tensor.matmul(
                                po0, mq[:, fo, 0:128], w2q[:, fo], start=st, stop=sp_
                            )
                            nc.tensor.matmul(
                                po1, mq[:, fo, 128:256], w2q[:, fo], start=st, stop=sp_
                            )
                        for gc2, po in ((0, po0), (1, po1)):
                            ot = op.tile([P, 512], F32, tag="ot")
                            nc.any.tensor_copy(ot, po)
                            g0 = (gp * 2 + gc2) * P
                            nc.sync.dma_start(
                                out[g0 : g0 + P, dq * 512 : (dq + 1) * 512], ot
                            )
    nc.compile()
    return nc


_NC_CACHE = None


def _get_nc():
    global _NC_CACHE
    if _NC_CACHE is None:
        _NC_CACHE = build_nc()
    return _NC_CACHE


def _in_maps(routed_in_egD, w1, w2, w3):
    x = np.ascontiguousarray(np.asarray(routed_in_egD, dtype=np.float32))
    w1 = np.ascontiguousarray(np.asarray(w1, dtype=np.float32))
    w2 = np.ascontiguousarray(np.asarray(w2, dtype=np.float32))
    w3 = np.ascontiguousarray(np.asarray(w3, dtype=np.float32))
    x_e = x.reshape(E, G, D)
    return [
        {"x": x_e[e], "w1": w1[e], "w2": w2[e], "w3": w3[e]} for e in range(E)
    ]


def kernel(routed_in_egD, w1, w2, w3):
    nc = _get_nc()
    res = run_bass_kernel_spmd(
        nc, _in_maps(routed_in_egD, w1, w2, w3), core_ids=list(range(E))
    )
    return np.concatenate([r["out"] for r in res.results], axis=0)


def run_traced(routed_in_egD, w1, w2, w3, **trace_kwargs):
    """For test.py: run with NTFF tracing; returns (full_out, BassKernelResults)."""
    nc = _get_nc()
    res = run_bass_kernel_spmd(
        nc,
        _in_maps(routed_in_egD, w1, w2, w3),
        core_ids=list(range(E)),
        trace=True,
        **trace_kwargs,
    )
    out = np.concatenate([r["out"] for r in res.results], axis=0)
    return out, res
